# revision 28
# baseline (speedup 1.0000x reference)
"""Trainium2 Bass kernel for nn_LocalOptLoss (batch 16384, data-parallel on 8 cores).

v5: tile_position-packed PE. Every per-sample matvec (blocks <= 32x32) runs as
4 concurrent 32x32 sub-array matmuls (one per lane), and up to 4 different
matvecs share one streaming pass via distinct (row_grp, col_grp) diagonals --
a strip-rotation assignment per quantity keeps concurrent matvecs on disjoint
sub-arrays while elementwise tiles stay 4-lane packed in partitions.
Biases ride the matmuls (ones-row in the input strips / full-width bias
matmuls that double as PSUM bank clears). tp/t2 tanh is one merged ACTIVATE
across two adjacent PSUM banks. The P_inv tail (s s^T u) plus norm/mean is
finished on host from three raw outputs (b, m = s*u, s). Junk matmuls on a
memset tile warm the PE HAM clock gate during the input DMA.
"""
import sys

sys.path.insert(0, "/opt/trn_rl_repo")

from contextlib import ExitStack

import numpy as np
from ml_dtypes import bfloat16 as np_bf16

import concourse.bass as bass
import concourse.bacc as bacc
import concourse.tile as tile
from concourse import mybir

N, NZ, OUT, H, B = 16, 16, 8, 32, 16384
R = 0.1
NCORES = 8
PER_CORE = B // NCORES          # 2048
LANES = 4
F = PER_CORE // LANES           # 512 cols per lane

F32 = mybir.dt.float32
BF16 = mybir.dt.bfloat16
AF = mybir.ActivationFunctionType
ALU = mybir.AluOpType

# ---- strip rotations per quantity (see header) ----
# gamma(a1)=0 d1(a2c1)=1 d2(a2c2)=2 alpha(th/hd/bk/...)=0 a3=0 eta(tf/q2)=3
# theta(s/u/b/m)=0 zeta(tp/q3)=3 beta(t2/q1b/q4/r)=2

# ---- weight pack layout: blocks replicated on 4 partition strips ----
_WCOL = {}
_CUR = [0]


def _wadd(name, cols):
    _WCOL[name] = _CUR[0]
    _CUR[0] += cols


for _n in ("pre1", "q1a", "q1n", "q3", "hd1", "pre2a", "th", "hd2n", "a3",
           "tf", "ppsi1", "ppsi2", "bk", "r"):
    _wadd(_n, 32)
for _n in ("s", "glin"):
    _wadd(_n, 32)                         # out padded to 32 (bank fully written)
for _n in ("f2", "tau2n"):
    _wadd(_n, 16)
for _n in ("zero", "btp", "bt2"):        # full-width strip-0 blocks
    _wadd(_n, 128)
_wadd("bias", 6)                          # 3 fp32 bias cols (bh1,bT1,bf1) bitcast
WCOLS = _CUR[0]


import os
KSTAGE = int(os.environ.get("KSTAGE", "99"))


def build_nc():
    nc = bacc.Bacc("TRN2", target_bir_lowering=False, debug=False,
                   num_devices=NCORES)
    xe_d = nc.dram_tensor("xe", [128, 2 * F], BF16, kind="ExternalInput")
    wt_d = nc.dram_tensor("wt", [128, WCOLS], BF16, kind="ExternalInput")
    out_d = nc.dram_tensor("vout", [128, 5 * F], BF16, kind="ExternalOutput")

    with tile.TileContext(nc) as tc, ExitStack() as ctx:
        sb = ctx.enter_context(tc.tile_pool(name="sb", bufs=1))
        ps = ctx.enter_context(tc.tile_pool(name="ps", bufs=1, space="PSUM"))

        wt = sb.tile([128, WCOLS], BF16, tag="wt", name="wt")
        xe = sb.tile([128, 2 * F], BF16, tag="xe", name="xe")
        junk_src = sb.tile([128, F], BF16, tag="jsrc", name="junk_src")
        a1 = sb.tile([128, F], BF16, tag="a1", name="a1")
        a2c1 = sb.tile([128, F], BF16, tag="a2c1", name="a2c1")
        a2c2 = sb.tile([128, F], BF16, tag="a2c2", name="a2c2")
        th = sb.tile([128, F], BF16, tag="th", name="th")
        a3 = sb.tile([128, F], BF16, tag="a3", name="a3")
        tf = sb.tile([128, F], BF16, tag="tf", name="tf")
        tpt2 = sb.tile([128, 2 * F], BF16, tag="tpt2", name="tpt2")
        sq_th = sb.tile([128, F], BF16, tag="sq_th", name="sq_th")
        sq_tf = sb.tile([128, F], BF16, tag="sq_tf", name="sq_tf")
        sq_tpt2 = sb.tile([128, 2 * F], BF16, tag="sq_tpt2", name="sq_tpt2")
        argh1 = sb.tile([128, F], BF16, tag="argh1", name="argh1")
        argh2 = sb.tile([128, F], BF16, tag="argh2", name="argh2")
        argA12 = sb.tile([128, 2 * F], BF16, tag="argA12", name="argA12")
        m2 = sb.tile([128, F], BF16, tag="m2", name="m2")
        argH = sb.tile([128, F], BF16, tag="argH", name="argH")
        argP = sb.tile([128, F], BF16, tag="argP", name="argP")
        argHP = sb.tile([128, F], BF16, tag="argHP", name="argHP")
        argF = sb.tile([128, F], BF16, tag="argF", name="argF")
        outt = sb.tile([128, 5 * F], BF16, tag="outt", name="outt")

        bias = {n: wt[:, _WCOL["bias"] + 2 * i:_WCOL["bias"] + 2 * i + 2]
                .bitcast(F32) for i, n in enumerate(("bh1", "bT1", "bf1"))}

        # PSUM: 4 rotating 1-bank slots (tag psA) + two 2-bank slots (big)
        junk_ps = ps.tile([128, 2 * F], F32, tag="big1", bufs=1, name="junk_ps")
        pre1 = ps.tile([128, F], F32, tag="psA", bufs=4, name="pre1")
        pre2c1 = ps.tile([128, F], F32, tag="psA", bufs=4, name="pre2c1")
        pre2c2 = ps.tile([128, F], F32, tag="psA", bufs=4, name="pre2c2")
        q31 = ps.tile([128, 2 * F], F32, tag="big2", bufs=1, name="q31")
        hd = ps.tile([128, F], F32, tag="psA", bufs=4, name="hd")
        thp = ps.tile([128, F], F32, tag="psA", bufs=4, name="thp")
        a3p = ps.tile([128, F], F32, tag="psA", bufs=4, name="a3p")
        tfp = ps.tile([128, F], F32, tag="psA", bufs=4, name="tfp")
        tpt2p = ps.tile([128, 2 * F], F32, tag="big1", bufs=1, name="tpt2p")
        bkp = ps.tile([128, F], F32, tag="psA", bufs=4, name="bkp")
        rp = ps.tile([128, F], F32, tag="psA", bufs=4, name="rp")
        q2p = ps.tile([128, F], F32, tag="psA", bufs=4, name="q2p")
        q4p = ps.tile([128, F], F32, tag="psA", bufs=4, name="q4p")
        sp = ps.tile([128, F], F32, tag="psA", bufs=4, name="sp")
        ub = ps.tile([128, 2 * F], F32, tag="big2", bufs=1, name="ub")

        # ---- input DMAs (HWDGE on sync + scalar queues) ----
        nc.sync.dma_start(out=xe, in_=xe_d.ap())
        nc.scalar.dma_start(out=wt, in_=wt_d.ap())


        x17 = lambda i: xe[32 * i:32 * i + 17, 0:F]
        x16 = lambda i: xe[32 * i:32 * i + 16, 0:F]
        e17 = lambda i: xe[32 * i:32 * i + 17, F:2 * F]
        e16 = lambda i: xe[32 * i:32 * i + 16, F:2 * F]

        def wblk(name, i, rows, cols):
            c = _WCOL[name]
            return wt[32 * i:32 * i + rows, c:c + cols]

        def mmv(out_tile, ocols, wname, rhs_fn, rows, rho_in, rho_out, out_p,
                start, stop):
            """One matvec = 4 concurrent sub-array matmuls (one per lane)."""
            for L in range(LANES):
                i = (L + rho_in) % LANES
                j = (L + rho_out) % LANES
                nc.tensor.matmul(
                    out_tile[32 * j:32 * j + out_p, ocols],
                    wblk(wname, i, rows, out_p), rhs_fn(i),
                    start=start, stop=stop, tile_position=(32 * i, 32 * j),
                    skip_group_check=True)

        def act_rhs(t, rho_in, c0=0):
            return lambda i: t[32 * i:32 * i + 32, c0:c0 + F]

        def zmm(out_tile, ocols, wname, start, stop):
            """Full-width strip-0 matmul: clears the bank, writes bias/zero."""
            nc.tensor.matmul(out_tile[0:128, ocols], wblk(wname, 0, 17, 128),
                             x17(0), start=start, stop=stop,
                             tile_position=(0, 0), skip_group_check=True)

        cF = slice(0, F)
        cB = slice(F, 2 * F)

        def fill(bank):
            # full-array junk MM into a bank that is overwritten later;
            # keeps the PE HAM window busy through act/DVE gaps
            nc.tensor.matmul(bank[0:128, 0:F], junk_src[:, 0:128], junk_src,
                             start=True, stop=True, tile_position=(0, 0),
                             skip_group_check=True)

        # ---- zero-clears for multi-writer banks ----
        zmm(pre2c1, cF, "zero", True, False)
        zmm(pre2c2, cF, "zero", True, False)
        zmm(hd, cF, "zero", True, False)

        # A1: pre1(fam0) q1a(1) q1b(2) q3(3)
        mmv(pre1, cF, "pre1", x17, 17, 0, 0, 32, True, True)
        mmv(pre2c1, cF, "q1a", e17, 17, 0, 1, 32, False, False)
        mmv(q31, cB, "q1n", e16, 16, 0, 2, 32, True, True)
        mmv(q31, cF, "q3", e16, 16, 0, 3, 32, True, True)
        # A2: hd1(0) q1a2(2)
        mmv(hd, cF, "hd1", x16, 16, 0, 0, 32, False, False)
        mmv(pre2c2, cF, "q1a", e17, 17, 0, 2, 32, False, False)

        nc.scalar.activation(a1, pre1, AF.Tanh)

        if KSTAGE < 2:
            nc.vector.tensor_copy(outt[:, 0:F], a1)
            nc.sync.dma_start(out=out_d.ap()[:, 0:F], in_=outt[:, 0:F])
            return _finish(nc)
        # B: pre2a -> c1(fam1), c2(fam2)
        mmv(pre2c1, cF, "pre2a", act_rhs(a1, 0), 32, 0, 1, 32, False, True)
        mmv(pre2c2, cF, "pre2a", act_rhs(a1, 0), 32, 0, 2, 32, False, True)

        nc.scalar.activation(a2c1, pre2c1, AF.Tanh)
        nc.scalar.activation(a2c2, pre2c2, AF.Tanh)

        if KSTAGE < 3:
            nc.vector.tensor_copy(outt[:, 0:F], a2c1)
            nc.sync.dma_start(out=out_d.ap()[:, 0:F], in_=outt[:, 0:F])
            return _finish(nc)
        # bias clears for tp/t2 halves
        zmm(tpt2p, cF, "btp", True, False)
        zmm(tpt2p, cB, "bt2", True, False)
        if KSTAGE < 32:
            nc.vector.tensor_copy(outt[:, 0:F], a2c2)
            nc.sync.dma_start(out=out_d.ap()[:, 0:F], in_=outt[:, 0:F])
            raise _Early

        # C1: th(3) a3(0) tf(1) hd2n(2)
        mmv(thp, cF, "th", act_rhs(a2c1, 0), 32, 1, 0, 32, True, True)
        mmv(a3p, cF, "a3", act_rhs(a2c1, 0), 32, 1, 0, 32, True, True)
        mmv(tfp, cF, "tf", act_rhs(a2c2, 0), 32, 2, 3, 32, True, True)
        mmv(hd, cF, "hd2n", act_rhs(a2c2, 0), 32, 2, 0, 32, False, True)

        _b = (lambda n: bias[n]) if KSTAGE >= 34 else (lambda n: 0.0)
        nc.scalar.activation(th, thp, AF.Tanh, bias=_b("bh1"))
        if KSTAGE < 33:
            nc.sync.dma_start(out=out_d.ap()[:, 0:F], in_=th)
            raise _Early
        nc.scalar.activation(a3, a3p, AF.Tanh, bias=_b("bT1"))
        nc.scalar.activation(tf, tfp, AF.Tanh, bias=_b("bf1"))

        nc.gpsimd.tensor_mul(sq_th, th, th)
        nc.vector.scalar_tensor_tensor(argh1, sq_th, -1.0, hd, ALU.add,
                                       ALU.mult)

        if KSTAGE < 40:
            nc.sync.dma_start(out=out_d.ap()[:, 0:F], in_=argh1)
            return _finish(nc)
        # D: ppsi1(2) ppsi2(3) t2m(1) bk(0)
        mmv(tpt2p, cF, "ppsi1", act_rhs(a3, 0), 32, 0, 3, 32, False, False)
        mmv(tpt2p, cF, "ppsi2", act_rhs(th, 0), 32, 0, 3, 32, False, True)
        mmv(tpt2p, cB, "pre2a", act_rhs(a3, 0), 32, 0, 2, 32, False, True)
        if KSTAGE >= 42:
            mmv(bkp, cF, "bk", act_rhs(argh1, 0), 32, 0, 0, 32, True, True)

        nc.scalar.activation(tpt2[:, F:2 * F], tpt2p[:, F:2 * F], AF.Tanh)
        nc.scalar.activation(tpt2[:, 0:F], tpt2p[:, 0:F], AF.Tanh)
        if KSTAGE < 42:
            nc.sync.dma_start(out=out_d.ap()[:, 0:F], in_=tpt2[:, 0:F])
            raise _Early
        nc.gpsimd.tensor_mul(sq_tf, tf, tf)
        nc.vector.tensor_mul(sq_tpt2[:, F:2 * F], tpt2[:, F:2 * F],
                             tpt2[:, F:2 * F])
        if KSTAGE < 43:
            nc.sync.dma_start(out=out_d.ap()[:, 0:F], in_=sq_tpt2[:, 0:F])
            raise _Early
        nc.vector.scalar_tensor_tensor(argA12[:, F:2 * F], sq_tpt2[:, F:2 * F],
                                       -1.0, q31[:, F:2 * F], ALU.add, ALU.mult)
        nc.vector.tensor_mul(sq_tpt2[:, 0:F], tpt2[:, 0:F], tpt2[:, 0:F])
        nc.vector.scalar_tensor_tensor(argA12[:, 0:F], sq_tpt2[:, 0:F],
                                       -1.0, q31[:, 0:F], ALU.add, ALU.mult)
        if KSTAGE < 44:
            nc.sync.dma_start(out=out_d.ap()[:, 0:F], in_=argA12[:, 0:F])
            raise _Early
        nc.vector.scalar_tensor_tensor(argh2, sq_th, -1.0, bkp, ALU.add,
                                       ALU.mult)
        nc.gpsimd.tensor_mul(m2, tpt2[:, F:2 * F], argA12[:, F:2 * F])

        if KSTAGE < 50:
            nc.vector.tensor_copy(outt[:, 0:F], argA12[:, 0:F])
            nc.sync.dma_start(out=out_d.ap()[:, 0:F], in_=outt[:, 0:F])
            return _finish(nc)
        # E1: r(3) q2(1) u1(0)
        mmv(rp, cF, "r", act_rhs(tpt2, 0), 32, 3, 2, 32, True, True)
        mmv(q2p, cF, "tf", act_rhs(argA12, 0, F), 32, 2, 3, 32, True, True)
        mmv(ub, cF, "glin", act_rhs(argh2, 0), 32, 0, 0, 32, True, True)

        nc.vector.scalar_tensor_tensor(outt[:, 3 * F:4 * F], sq_tf, -1.0,
                                       q2p, ALU.add, ALU.mult)
        nc.vector.tensor_mul(argH, m2, rp)

        # C2: s(3)
        mmv(sp, cF, "s", act_rhs(a2c1, 0), 32, 1, 0, 32, True, True)
        nc.scalar.activation(outt[:, 2 * F:3 * F], sp, AF.Tanh)
        nc.sync.dma_start(out=out_d.ap()[:, 2 * F:3 * F],
                          in_=outt[:, 2 * F:3 * F])

        if KSTAGE < 60:
            nc.sync.dma_start(out=out_d.ap()[:, F:2 * F], in_=outt[:, F:2 * F])
            return _finish(nc)
        # E2: q4(3)
        mmv(q4p, cF, "r", act_rhs(argA12, 0), 32, 3, 2, 32, True, True)

        nc.vector.scalar_tensor_tensor(argP, sq_tpt2[:, F:2 * F], -1.0, q4p,
                                       ALU.add, ALU.mult)
        nc.vector.scalar_tensor_tensor(outt[:, 4 * F:5 * F], argH, 2.0, argP,
                                       ALU.mult, ALU.add)
        nc.sync.dma_start(out=out_d.ap()[:, 3 * F:5 * F],
                          in_=outt[:, 3 * F:5 * F])
        nc.vector.tensor_copy(outt[:, 0:F], ub[:, 0:F])
        nc.vector.tensor_mul(outt[:, F:2 * F], outt[:, 2 * F:3 * F], ub[:, 0:F])
        nc.sync.dma_start(out=out_d.ap()[:, 0:2 * F], in_=outt[:, 0:2 * F])


    nc.compile()
    return nc


def _finish(nc):
    return nc


def _host_weights(Wf1, bf1, Wf2, Wh1, bh1, Wh2, WT1, bT1, WT2,
                  Wtau1, btau1, Wtau2, Wpsi1, bpsi1, Wpsi2, WP):
    f = np.float64
    A = lambda a: np.asarray(a, f)
    Wf1, bf1, Wf2 = A(Wf1), A(bf1), A(Wf2)
    Wh1, bh1, Wh2 = A(Wh1), A(bh1), A(Wh2)
    WT1, bT1, WT2 = A(WT1), A(bT1), A(WT2)
    Wtau1, btau1, Wtau2 = A(Wtau1), A(btau1), A(Wtau2)
    Wpsi1, bpsi1, Wpsi2, WP = A(Wpsi1), A(bpsi1), A(Wpsi2), A(WP)
    Wpsi1z, Wpsi1y = Wpsi1[:, :NZ], Wpsi1[:, NZ:]

    def wb(WT_, b=None):                      # lhsT block (+ bias row)
        blk = WT_
        if b is not None:
            blk = np.concatenate([WT_, b.reshape(1, -1)], axis=0)
        return blk

    blocks = {
        "pre1": wb(WT1.T, bT1), "q1a": wb(Wtau1.T, btau1), "q1n": Wtau1.T,
        "q3": Wpsi1z.T, "hd1": Wh1.T, "pre2a": (Wtau1 @ WT2).T,
        "th": (Wh1 @ Wtau2).T, "hd2n": -(Wh1 @ Wtau2).T,
        "a3": (WT1 @ Wtau2).T, "tf": (Wf1 @ Wtau2).T,
        "ppsi1": (Wpsi1z @ WT2).T, "ppsi2": (Wpsi1y @ Wh2).T,
        "bk": Wh2.T @ Wh2 / R, "r": (Wtau1 @ Wpsi2).T,
        "s": np.pad((WP @ Wtau2).T, ((0, 0), (0, 16))),
        "glin": np.pad(Wh1, ((0, 0), (0, 16))),
        "f2": Wf2.T, "tau2n": -Wtau2.T,
    }
    pack = np.zeros((128, WCOLS), np.float32)
    for name, blk in blocks.items():
        c = _WCOL[name]
        rr, cc = blk.shape
        for L in range(LANES):
            pack[32 * L:32 * L + rr, c:c + cc] = blk
    # full-width strip-0 blocks: rows 0-15 zero, row 16 = tiled bias
    for name, bvec in (("zero", np.zeros(128)), ("btp", np.tile(bpsi1, 4)),
                       ("bt2", np.tile(btau1, 4))):
        pack[16, _WCOL[name]:_WCOL[name] + 128] = bvec
    packb = pack.astype(np_bf16)
    for i, bvec in enumerate((bh1, bT1, bf1)):
        col = np.tile(np.asarray(bvec, np.float32), LANES).reshape(128, 1)
        packb[:, _WCOL["bias"] + 2 * i:_WCOL["bias"] + 2 * i + 2] = \
            col.view(np_bf16)
    return packb


_CACHE = {}
_HOST = {}


def _get_nc():
    if "nc" not in _CACHE:
        _CACHE["nc"] = build_nc()
    return _CACHE["nc"]


def _in_maps(x_batch, e_batch, wts):
    wpack = _host_weights(**wts)
    _HOST["Wf2"] = np.asarray(wts["Wf2"], np.float64)
    _HOST["Wtau2"] = np.asarray(wts["Wtau2"], np.float64)

    def strips(a, rows):
        # (2048, rows) -> [128, 512] with lane L at partitions 32L..32L+rows,
        # row 16 = ones, rest zero
        out = np.zeros((128, F), np.float32)
        a = np.asarray(a, np.float32).reshape(LANES, F, rows)
        for L in range(LANES):
            out[32 * L:32 * L + rows] = a[L].T
            out[32 * L + 16] = 1.0
        return out

    in_maps = []
    for c in range(NCORES):
        cs = slice(c * PER_CORE, (c + 1) * PER_CORE)
        xeb = np.concatenate([strips(x_batch[cs], N), strips(e_batch[cs], NZ)],
                             axis=1).astype(np_bf16)
        in_maps.append({"xe": np.ascontiguousarray(xeb), "wt": wpack})
    return in_maps


def _reduce(results):
    Wf2, Wtau2 = _HOST["Wf2"], _HOST["Wtau2"]
    total = np.float64(0.0)
    for r in results:
        o = np.asarray(r["vout"], np.float64)      # (128, 2560)
        o = o.reshape(LANES, 32, 5 * F)
        u = o[:, 0:16, 0:F]
        m = o[:, 0:16, F:2 * F]
        s = o[:, 0:16, 2 * F:3 * F]
        aF = o[:, :, 3 * F:4 * F]                  # (L, 32, F)
        aHP = o[:, :, 4 * F:5 * F]
        v = (u + s * m.sum(axis=1, keepdims=True)
             + np.einsum('nh,lhc->lnc', Wf2, aF)
             - np.einsum('nh,lhc->lnc', Wtau2, aHP))
        total += np.sqrt((v * v).sum(axis=1)).sum()
    return np.asarray(total / B, dtype=np.float32)


def kernel(x_batch, e_batch, **wts):
    from concourse.bass_utils import run_bass_kernel_spmd
    nc = _get_nc()
    in_maps = _in_maps(np.asarray(x_batch, np.float32),
                       np.asarray(e_batch, np.float32), wts)
    res = run_bass_kernel_spmd(nc, in_maps, core_ids=list(range(NCORES)))
    return _reduce(res.results)


if __name__ == "__main__":
    rng = np.random.default_rng(0)
    wts = {
        "Wf1": rng.normal(size=(H, N)) * .3, "bf1": rng.normal(size=(H,)) * .3,
        "Wf2": rng.normal(size=(N, H)) * .3,
        "Wh1": rng.normal(size=(H, N)) * .3, "bh1": rng.normal(size=(H,)) * .3,
        "Wh2": rng.normal(size=(OUT, H)) * .3,
        "WT1": rng.normal(size=(H, N)) * .3, "bT1": rng.normal(size=(H,)) * .3,
        "WT2": rng.normal(size=(NZ, H)) * .3,
        "Wtau1": rng.normal(size=(H, NZ)) * .3,
        "btau1": rng.normal(size=(H,)) * .3,
        "Wtau2": rng.normal(size=(N, H)) * .3,
        "Wpsi1": rng.normal(size=(H, NZ + OUT)) * .3,
        "bpsi1": rng.normal(size=(H,)) * .3,
        "Wpsi2": rng.normal(size=(NZ, H)) * .3,
        "WP": rng.normal(size=(N, N)) * .3,
    }
    x = rng.normal(size=(B, N)).astype(np.float32)
    e = (rng.normal(size=(B, NZ)) * 0.1).astype(np.float32)
    print(kernel(x, e, **{k: np.asarray(v, np.float32)
                          for k, v in wts.items()}))


# revision 29
# speedup vs baseline: 1.0802x; 1.0802x over previous
"""Trainium2 Bass kernel for nn_LocalOptLoss (batch 16384, data-parallel on 8 cores).

v5: tile_position-packed PE. Every per-sample matvec (blocks <= 32x32) runs as
4 concurrent 32x32 sub-array matmuls (one per lane), and up to 4 different
matvecs share one streaming pass via distinct (row_grp, col_grp) diagonals --
a strip-rotation assignment per quantity keeps concurrent matvecs on disjoint
sub-arrays while elementwise tiles stay 4-lane packed in partitions.
Biases ride the matmuls (ones-row in the input strips / full-width bias
matmuls that double as PSUM bank clears). tp/t2 tanh is one merged ACTIVATE
across two adjacent PSUM banks. The P_inv tail (s s^T u) plus norm/mean is
finished on host from three raw outputs (b, m = s*u, s). Junk matmuls on a
memset tile warm the PE HAM clock gate during the input DMA.
"""
import sys

sys.path.insert(0, "/opt/trn_rl_repo")

from contextlib import ExitStack

import numpy as np
from ml_dtypes import bfloat16 as np_bf16

import concourse.bass as bass
import concourse.bacc as bacc
import concourse.tile as tile
from concourse import mybir

N, NZ, OUT, H, B = 16, 16, 8, 32, 16384
R = 0.1
NCORES = 8
PER_CORE = B // NCORES          # 2048
LANES = 4
F = PER_CORE // LANES           # 512 cols per lane

F32 = mybir.dt.float32
BF16 = mybir.dt.bfloat16
AF = mybir.ActivationFunctionType
ALU = mybir.AluOpType

# ---- strip rotations per quantity (see header) ----
# gamma(a1)=0 d1(a2c1)=1 d2(a2c2)=2 alpha(th/hd/bk/...)=0 a3=0 eta(tf/q2)=3
# theta(s/u/b/m)=0 zeta(tp/q3)=3 beta(t2/q1b/q4/r)=2

# ---- weight pack layout: blocks replicated on 4 partition strips ----
_WCOL = {}
_CUR = [0]


def _wadd(name, cols):
    _WCOL[name] = _CUR[0]
    _CUR[0] += cols


for _n in ("pre1", "q1a", "q1n", "q3", "hd1", "pre2a", "th", "hd2n", "a3",
           "tf", "ppsi1", "ppsi2", "bk", "r"):
    _wadd(_n, 32)
for _n in ("s", "glin"):
    _wadd(_n, 32)                         # out padded to 32 (bank fully written)
for _n in ("f2", "tau2n"):
    _wadd(_n, 16)
for _n in ("zero", "btp", "bt2"):        # full-width strip-0 blocks
    _wadd(_n, 128)
_wadd("bias", 6)                          # 3 fp32 bias cols (bh1,bT1,bf1) bitcast
WCOLS = _CUR[0]


import os
KSTAGE = int(os.environ.get("KSTAGE", "99"))


def build_nc():
    nc = bacc.Bacc("TRN2", target_bir_lowering=False, debug=False,
                   num_devices=NCORES)
    xe_d = nc.dram_tensor("xe", [128, 2 * F], BF16, kind="ExternalInput")
    wt_d = nc.dram_tensor("wt", [128, WCOLS], BF16, kind="ExternalInput")
    out_d = nc.dram_tensor("vout", [128, 5 * F], BF16, kind="ExternalOutput")

    with tile.TileContext(nc) as tc, ExitStack() as ctx:
        sb = ctx.enter_context(tc.tile_pool(name="sb", bufs=1))
        ps = ctx.enter_context(tc.tile_pool(name="ps", bufs=1, space="PSUM"))

        wt = sb.tile([128, WCOLS], BF16, tag="wt", name="wt")
        xe = sb.tile([128, 2 * F], BF16, tag="xe", name="xe")
        junk_src = sb.tile([128, F], BF16, tag="jsrc", name="junk_src")
        a1 = sb.tile([128, F], BF16, tag="a1", name="a1")
        a2c1 = sb.tile([128, F], BF16, tag="a2c1", name="a2c1")
        a2c2 = sb.tile([128, F], BF16, tag="a2c2", name="a2c2")
        th = sb.tile([128, F], BF16, tag="th", name="th")
        a3 = sb.tile([128, F], BF16, tag="a3", name="a3")
        tf = sb.tile([128, F], BF16, tag="tf", name="tf")
        tpt2 = sb.tile([128, 2 * F], BF16, tag="tpt2", name="tpt2")
        sq_th = sb.tile([128, F], BF16, tag="sq_th", name="sq_th")
        sq_tf = sb.tile([128, F], BF16, tag="sq_tf", name="sq_tf")
        sq_tpt2 = sb.tile([128, 2 * F], BF16, tag="sq_tpt2", name="sq_tpt2")
        argh1 = sb.tile([128, F], BF16, tag="argh1", name="argh1")
        argh2 = sb.tile([128, F], BF16, tag="argh2", name="argh2")
        argA12 = sb.tile([128, 2 * F], BF16, tag="argA12", name="argA12")
        m2 = sb.tile([128, F], BF16, tag="m2", name="m2")
        argH = sb.tile([128, F], BF16, tag="argH", name="argH")
        argP = sb.tile([128, F], BF16, tag="argP", name="argP")
        argHP = sb.tile([128, F], BF16, tag="argHP", name="argHP")
        argF = sb.tile([128, F], BF16, tag="argF", name="argF")
        outt = sb.tile([128, 5 * F], BF16, tag="outt", name="outt")

        bias = {n: wt[:, _WCOL["bias"] + 2 * i:_WCOL["bias"] + 2 * i + 2]
                .bitcast(F32) for i, n in enumerate(("bh1", "bT1", "bf1"))}

        # PSUM: 4 rotating 1-bank slots (tag psA) + two 2-bank slots (big)
        junk_ps = ps.tile([128, 2 * F], F32, tag="big1", bufs=1, name="junk_ps")
        pre1 = ps.tile([128, F], F32, tag="psA", bufs=4, name="pre1")
        pre2c1 = ps.tile([128, F], F32, tag="psA", bufs=4, name="pre2c1")
        pre2c2 = ps.tile([128, F], F32, tag="psA", bufs=4, name="pre2c2")
        q31 = ps.tile([128, 2 * F], F32, tag="big2", bufs=1, name="q31")
        hd = ps.tile([128, F], F32, tag="psA", bufs=4, name="hd")
        thp = ps.tile([128, F], F32, tag="psA", bufs=4, name="thp")
        a3p = ps.tile([128, F], F32, tag="psA", bufs=4, name="a3p")
        tfp = ps.tile([128, F], F32, tag="psA", bufs=4, name="tfp")
        tpt2p = ps.tile([128, 2 * F], F32, tag="big1", bufs=1, name="tpt2p")
        bkp = ps.tile([128, F], F32, tag="psA", bufs=4, name="bkp")
        rp = ps.tile([128, F], F32, tag="psA", bufs=4, name="rp")
        q2p = ps.tile([128, F], F32, tag="psA", bufs=4, name="q2p")
        q4p = ps.tile([128, F], F32, tag="psA", bufs=4, name="q4p")
        sp = ps.tile([128, F], F32, tag="psA", bufs=4, name="sp")
        ub = ps.tile([128, 2 * F], F32, tag="big2", bufs=1, name="ub")

        # ---- input DMAs (HWDGE on sync + scalar queues) ----
        nc.sync.dma_start(out=xe, in_=xe_d.ap())
        nc.scalar.dma_start(out=wt, in_=wt_d.ap())


        x17 = lambda i: xe[32 * i:32 * i + 17, 0:F]
        x16 = lambda i: xe[32 * i:32 * i + 16, 0:F]
        e17 = lambda i: xe[32 * i:32 * i + 17, F:2 * F]
        e16 = lambda i: xe[32 * i:32 * i + 16, F:2 * F]

        def wblk(name, i, rows, cols):
            c = _WCOL[name]
            return wt[32 * i:32 * i + rows, c:c + cols]

        def mmv(out_tile, ocols, wname, rhs_fn, rows, rho_in, rho_out, out_p,
                start, stop):
            """One matvec = 4 concurrent sub-array matmuls (one per lane)."""
            for L in range(LANES):
                i = (L + rho_in) % LANES
                j = (L + rho_out) % LANES
                nc.tensor.matmul(
                    out_tile[32 * j:32 * j + out_p, ocols],
                    wblk(wname, i, rows, out_p), rhs_fn(i),
                    start=start, stop=stop, tile_position=(32 * i, 32 * j),
                    skip_group_check=True)

        def act_rhs(t, rho_in, c0=0):
            return lambda i: t[32 * i:32 * i + 32, c0:c0 + F]

        def zmm(out_tile, ocols, wname, start, stop):
            """Full-width strip-0 matmul: clears the bank, writes bias/zero."""
            nc.tensor.matmul(out_tile[0:128, ocols], wblk(wname, 0, 17, 128),
                             x17(0), start=start, stop=stop,
                             tile_position=(0, 0), skip_group_check=True)

        cF = slice(0, F)
        cB = slice(F, 2 * F)

        def fill(bank):
            # full-array junk MM into a bank that is overwritten later;
            # keeps the PE HAM window busy through act/DVE gaps
            nc.tensor.matmul(bank[0:128, 0:F], junk_src[:, 0:128], junk_src,
                             start=True, stop=True, tile_position=(0, 0),
                             skip_group_check=True)

        # A1: pre1(fam0) q1b(2) q3(3) first -- critical chain head
        mmv(pre1, cF, "pre1", x17, 17, 0, 0, 32, True, True)
        mmv(q31, cB, "q1n", e16, 16, 0, 2, 32, True, True)
        mmv(q31, cF, "q3", e16, 16, 0, 3, 32, True, True)
        # zero-clears for multi-writer banks (off critical path)
        zmm(pre2c1, cF, "zero", True, False)
        zmm(pre2c2, cF, "zero", True, False)
        zmm(hd, cF, "zero", True, False)
        # A2: q1a(1) q1a2(2) hd1(0)
        mmv(pre2c1, cF, "q1a", e17, 17, 0, 1, 32, False, False)
        mmv(pre2c2, cF, "q1a", e17, 17, 0, 2, 32, False, False)
        mmv(hd, cF, "hd1", x16, 16, 0, 0, 32, False, False)

        nc.scalar.activation(a1, pre1, AF.Tanh)

        if KSTAGE < 2:
            nc.vector.tensor_copy(outt[:, 0:F], a1)
            nc.sync.dma_start(out=out_d.ap()[:, 0:F], in_=outt[:, 0:F])
            return _finish(nc)
        # B: pre2a -> c1(fam1), c2(fam2)
        mmv(pre2c1, cF, "pre2a", act_rhs(a1, 0), 32, 0, 1, 32, False, True)
        mmv(pre2c2, cF, "pre2a", act_rhs(a1, 0), 32, 0, 2, 32, False, True)

        nc.scalar.activation(a2c1, pre2c1, AF.Tanh)
        nc.scalar.activation(a2c2, pre2c2, AF.Tanh)

        if KSTAGE < 3:
            nc.vector.tensor_copy(outt[:, 0:F], a2c1)
            nc.sync.dma_start(out=out_d.ap()[:, 0:F], in_=outt[:, 0:F])
            return _finish(nc)
        # bias clears for tp/t2 halves
        zmm(tpt2p, cF, "btp", True, False)
        zmm(tpt2p, cB, "bt2", True, False)
        if KSTAGE < 32:
            nc.vector.tensor_copy(outt[:, 0:F], a2c2)
            nc.sync.dma_start(out=out_d.ap()[:, 0:F], in_=outt[:, 0:F])
            raise _Early

        # C1: th(3) a3(0) tf(1) hd2n(2)
        mmv(thp, cF, "th", act_rhs(a2c1, 0), 32, 1, 0, 32, True, True)
        mmv(a3p, cF, "a3", act_rhs(a2c1, 0), 32, 1, 0, 32, True, True)
        mmv(tfp, cF, "tf", act_rhs(a2c2, 0), 32, 2, 3, 32, True, True)
        mmv(hd, cF, "hd2n", act_rhs(a2c2, 0), 32, 2, 0, 32, False, True)

        _b = (lambda n: bias[n]) if KSTAGE >= 34 else (lambda n: 0.0)
        nc.scalar.activation(th, thp, AF.Tanh, bias=_b("bh1"))
        if KSTAGE < 33:
            nc.sync.dma_start(out=out_d.ap()[:, 0:F], in_=th)
            raise _Early
        nc.scalar.activation(a3, a3p, AF.Tanh, bias=_b("bT1"))
        nc.scalar.activation(tf, tfp, AF.Tanh, bias=_b("bf1"))

        nc.gpsimd.tensor_mul(sq_th, th, th)
        nc.vector.scalar_tensor_tensor(argh1, sq_th, -1.0, hd, ALU.add,
                                       ALU.mult)

        if KSTAGE < 40:
            nc.sync.dma_start(out=out_d.ap()[:, 0:F], in_=argh1)
            return _finish(nc)
        # D: ppsi1(2) ppsi2(3) t2m(1) bk(0)
        mmv(tpt2p, cF, "ppsi1", act_rhs(a3, 0), 32, 0, 3, 32, False, False)
        mmv(tpt2p, cF, "ppsi2", act_rhs(th, 0), 32, 0, 3, 32, False, True)
        mmv(tpt2p, cB, "pre2a", act_rhs(a3, 0), 32, 0, 2, 32, False, True)
        if KSTAGE >= 42:
            mmv(bkp, cF, "bk", act_rhs(argh1, 0), 32, 0, 0, 32, True, True)

        nc.scalar.activation(tpt2[:, F:2 * F], tpt2p[:, F:2 * F], AF.Tanh)
        nc.scalar.activation(tpt2[:, 0:F], tpt2p[:, 0:F], AF.Tanh)
        if KSTAGE < 42:
            nc.sync.dma_start(out=out_d.ap()[:, 0:F], in_=tpt2[:, 0:F])
            raise _Early
        nc.gpsimd.tensor_mul(sq_tf, tf, tf)
        nc.vector.tensor_mul(sq_tpt2[:, F:2 * F], tpt2[:, F:2 * F],
                             tpt2[:, F:2 * F])
        if KSTAGE < 43:
            nc.sync.dma_start(out=out_d.ap()[:, 0:F], in_=sq_tpt2[:, 0:F])
            raise _Early
        nc.vector.scalar_tensor_tensor(argA12[:, F:2 * F], sq_tpt2[:, F:2 * F],
                                       -1.0, q31[:, F:2 * F], ALU.add, ALU.mult)
        nc.vector.tensor_mul(sq_tpt2[:, 0:F], tpt2[:, 0:F], tpt2[:, 0:F])
        nc.vector.scalar_tensor_tensor(argA12[:, 0:F], sq_tpt2[:, 0:F],
                                       -1.0, q31[:, 0:F], ALU.add, ALU.mult)
        if KSTAGE < 44:
            nc.sync.dma_start(out=out_d.ap()[:, 0:F], in_=argA12[:, 0:F])
            raise _Early
        nc.vector.scalar_tensor_tensor(argh2, sq_th, -1.0, bkp, ALU.add,
                                       ALU.mult)
        nc.gpsimd.tensor_mul(m2, tpt2[:, F:2 * F], argA12[:, F:2 * F])

        if KSTAGE < 50:
            nc.vector.tensor_copy(outt[:, 0:F], argA12[:, 0:F])
            nc.sync.dma_start(out=out_d.ap()[:, 0:F], in_=outt[:, 0:F])
            return _finish(nc)
        # E1: r(3) q2(1) u1(0)
        mmv(rp, cF, "r", act_rhs(tpt2, 0), 32, 3, 2, 32, True, True)
        mmv(q2p, cF, "tf", act_rhs(argA12, 0, F), 32, 2, 3, 32, True, True)
        mmv(ub, cF, "glin", act_rhs(argh2, 0), 32, 0, 0, 32, True, True)

        nc.vector.scalar_tensor_tensor(outt[:, 3 * F:4 * F], sq_tf, -1.0,
                                       q2p, ALU.add, ALU.mult)
        nc.vector.tensor_mul(argH, m2, rp)

        # C2: s(3)
        mmv(sp, cF, "s", act_rhs(a2c1, 0), 32, 1, 0, 32, True, True)
        nc.scalar.activation(outt[:, 2 * F:3 * F], sp, AF.Tanh)
        nc.sync.dma_start(out=out_d.ap()[:, 2 * F:3 * F],
                          in_=outt[:, 2 * F:3 * F])

        if KSTAGE < 60:
            nc.sync.dma_start(out=out_d.ap()[:, F:2 * F], in_=outt[:, F:2 * F])
            return _finish(nc)
        # E2: q4(3)
        mmv(q4p, cF, "r", act_rhs(argA12, 0), 32, 3, 2, 32, True, True)

        nc.vector.scalar_tensor_tensor(argP, sq_tpt2[:, F:2 * F], -1.0, q4p,
                                       ALU.add, ALU.mult)
        nc.vector.scalar_tensor_tensor(outt[:, 4 * F:5 * F], argH, 2.0, argP,
                                       ALU.mult, ALU.add)
        nc.sync.dma_start(out=out_d.ap()[:, 3 * F:5 * F],
                          in_=outt[:, 3 * F:5 * F])
        nc.vector.tensor_copy(outt[:, 0:F], ub[:, 0:F])
        nc.vector.tensor_mul(outt[:, F:2 * F], outt[:, 2 * F:3 * F], ub[:, 0:F])
        nc.sync.dma_start(out=out_d.ap()[:, 0:2 * F], in_=outt[:, 0:2 * F])


    nc.compile()
    return nc


def _finish(nc):
    return nc


def _host_weights(Wf1, bf1, Wf2, Wh1, bh1, Wh2, WT1, bT1, WT2,
                  Wtau1, btau1, Wtau2, Wpsi1, bpsi1, Wpsi2, WP):
    f = np.float64
    A = lambda a: np.asarray(a, f)
    Wf1, bf1, Wf2 = A(Wf1), A(bf1), A(Wf2)
    Wh1, bh1, Wh2 = A(Wh1), A(bh1), A(Wh2)
    WT1, bT1, WT2 = A(WT1), A(bT1), A(WT2)
    Wtau1, btau1, Wtau2 = A(Wtau1), A(btau1), A(Wtau2)
    Wpsi1, bpsi1, Wpsi2, WP = A(Wpsi1), A(bpsi1), A(Wpsi2), A(WP)
    Wpsi1z, Wpsi1y = Wpsi1[:, :NZ], Wpsi1[:, NZ:]

    def wb(WT_, b=None):                      # lhsT block (+ bias row)
        blk = WT_
        if b is not None:
            blk = np.concatenate([WT_, b.reshape(1, -1)], axis=0)
        return blk

    blocks = {
        "pre1": wb(WT1.T, bT1), "q1a": wb(Wtau1.T, btau1), "q1n": Wtau1.T,
        "q3": Wpsi1z.T, "hd1": Wh1.T, "pre2a": (Wtau1 @ WT2).T,
        "th": (Wh1 @ Wtau2).T, "hd2n": -(Wh1 @ Wtau2).T,
        "a3": (WT1 @ Wtau2).T, "tf": (Wf1 @ Wtau2).T,
        "ppsi1": (Wpsi1z @ WT2).T, "ppsi2": (Wpsi1y @ Wh2).T,
        "bk": Wh2.T @ Wh2 / R, "r": (Wtau1 @ Wpsi2).T,
        "s": np.pad((WP @ Wtau2).T, ((0, 0), (0, 16))),
        "glin": np.pad(Wh1, ((0, 0), (0, 16))),
        "f2": Wf2.T, "tau2n": -Wtau2.T,
    }
    pack = np.zeros((128, WCOLS), np.float32)
    for name, blk in blocks.items():
        c = _WCOL[name]
        rr, cc = blk.shape
        for L in range(LANES):
            pack[32 * L:32 * L + rr, c:c + cc] = blk
    # full-width strip-0 blocks: rows 0-15 zero, row 16 = tiled bias
    for name, bvec in (("zero", np.zeros(128)), ("btp", np.tile(bpsi1, 4)),
                       ("bt2", np.tile(btau1, 4))):
        pack[16, _WCOL[name]:_WCOL[name] + 128] = bvec
    packb = pack.astype(np_bf16)
    for i, bvec in enumerate((bh1, bT1, bf1)):
        col = np.tile(np.asarray(bvec, np.float32), LANES).reshape(128, 1)
        packb[:, _WCOL["bias"] + 2 * i:_WCOL["bias"] + 2 * i + 2] = \
            col.view(np_bf16)
    return packb


_CACHE = {}
_HOST = {}


def _get_nc():
    if "nc" not in _CACHE:
        _CACHE["nc"] = build_nc()
    return _CACHE["nc"]


def _in_maps(x_batch, e_batch, wts):
    wpack = _host_weights(**wts)
    _HOST["Wf2"] = np.asarray(wts["Wf2"], np.float64)
    _HOST["Wtau2"] = np.asarray(wts["Wtau2"], np.float64)

    def strips(a, rows):
        # (2048, rows) -> [128, 512] with lane L at partitions 32L..32L+rows,
        # row 16 = ones, rest zero
        out = np.zeros((128, F), np.float32)
        a = np.asarray(a, np.float32).reshape(LANES, F, rows)
        for L in range(LANES):
            out[32 * L:32 * L + rows] = a[L].T
            out[32 * L + 16] = 1.0
        return out

    in_maps = []
    for c in range(NCORES):
        cs = slice(c * PER_CORE, (c + 1) * PER_CORE)
        xeb = np.concatenate([strips(x_batch[cs], N), strips(e_batch[cs], NZ)],
                             axis=1).astype(np_bf16)
        in_maps.append({"xe": np.ascontiguousarray(xeb), "wt": wpack})
    return in_maps


def _reduce(results):
    Wf2, Wtau2 = _HOST["Wf2"], _HOST["Wtau2"]
    total = np.float64(0.0)
    for r in results:
        o = np.asarray(r["vout"], np.float64)      # (128, 2560)
        o = o.reshape(LANES, 32, 5 * F)
        u = o[:, 0:16, 0:F]
        m = o[:, 0:16, F:2 * F]
        s = o[:, 0:16, 2 * F:3 * F]
        aF = o[:, :, 3 * F:4 * F]                  # (L, 32, F)
        aHP = o[:, :, 4 * F:5 * F]
        v = (u + s * m.sum(axis=1, keepdims=True)
             + np.einsum('nh,lhc->lnc', Wf2, aF)
             - np.einsum('nh,lhc->lnc', Wtau2, aHP))
        total += np.sqrt((v * v).sum(axis=1)).sum()
    return np.asarray(total / B, dtype=np.float32)


def kernel(x_batch, e_batch, **wts):
    from concourse.bass_utils import run_bass_kernel_spmd
    nc = _get_nc()
    in_maps = _in_maps(np.asarray(x_batch, np.float32),
                       np.asarray(e_batch, np.float32), wts)
    res = run_bass_kernel_spmd(nc, in_maps, core_ids=list(range(NCORES)))
    return _reduce(res.results)


if __name__ == "__main__":
    rng = np.random.default_rng(0)
    wts = {
        "Wf1": rng.normal(size=(H, N)) * .3, "bf1": rng.normal(size=(H,)) * .3,
        "Wf2": rng.normal(size=(N, H)) * .3,
        "Wh1": rng.normal(size=(H, N)) * .3, "bh1": rng.normal(size=(H,)) * .3,
        "Wh2": rng.normal(size=(OUT, H)) * .3,
        "WT1": rng.normal(size=(H, N)) * .3, "bT1": rng.normal(size=(H,)) * .3,
        "WT2": rng.normal(size=(NZ, H)) * .3,
        "Wtau1": rng.normal(size=(H, NZ)) * .3,
        "btau1": rng.normal(size=(H,)) * .3,
        "Wtau2": rng.normal(size=(N, H)) * .3,
        "Wpsi1": rng.normal(size=(H, NZ + OUT)) * .3,
        "bpsi1": rng.normal(size=(H,)) * .3,
        "Wpsi2": rng.normal(size=(NZ, H)) * .3,
        "WP": rng.normal(size=(N, N)) * .3,
    }
    x = rng.normal(size=(B, N)).astype(np.float32)
    e = (rng.normal(size=(B, NZ)) * 0.1).astype(np.float32)
    print(kernel(x, e, **{k: np.asarray(v, np.float32)
                          for k, v in wts.items()}))


# revision 30
# speedup vs baseline: 1.2110x; 1.1211x over previous
"""Trainium2 Bass kernel for nn_LocalOptLoss (batch 16384, data-parallel on 8 cores).

v5: tile_position-packed PE. Every per-sample matvec (blocks <= 32x32) runs as
4 concurrent 32x32 sub-array matmuls (one per lane), and up to 4 different
matvecs share one streaming pass via distinct (row_grp, col_grp) diagonals --
a strip-rotation assignment per quantity keeps concurrent matvecs on disjoint
sub-arrays while elementwise tiles stay 4-lane packed in partitions.
Biases ride the matmuls (ones-row in the input strips / full-width bias
matmuls that double as PSUM bank clears). tp/t2 tanh is one merged ACTIVATE
across two adjacent PSUM banks. The P_inv tail (s s^T u) plus norm/mean is
finished on host from three raw outputs (b, m = s*u, s). Junk matmuls on a
memset tile warm the PE HAM clock gate during the input DMA.
"""
import sys

sys.path.insert(0, "/opt/trn_rl_repo")

from contextlib import ExitStack

import numpy as np
from ml_dtypes import bfloat16 as np_bf16

import concourse.bass as bass
import concourse.bacc as bacc
import concourse.tile as tile
from concourse import mybir

N, NZ, OUT, H, B = 16, 16, 8, 32, 16384
R = 0.1
NCORES = 8
PER_CORE = B // NCORES          # 2048
LANES = 4
F = PER_CORE // LANES           # 512 cols per lane

F32 = mybir.dt.float32
BF16 = mybir.dt.bfloat16
AF = mybir.ActivationFunctionType
ALU = mybir.AluOpType

# ---- strip rotations per quantity (see header) ----
# gamma(a1)=0 d1(a2c1)=1 d2(a2c2)=2 alpha(th/hd/bk/...)=0 a3=0 eta(tf/q2)=3
# theta(s/u/b/m)=0 zeta(tp/q3)=3 beta(t2/q1b/q4/r)=2

# ---- weight pack layout: blocks replicated on 4 partition strips ----
_WCOL = {}
_CUR = [0]


def _wadd(name, cols):
    _WCOL[name] = _CUR[0]
    _CUR[0] += cols


for _n in ("pre1", "q1a", "q1n", "q3", "hd1", "pre2a", "th", "hd2n", "a3",
           "tf", "ppsi1", "ppsi2", "bk", "r"):
    _wadd(_n, 32)
for _n in ("s", "glin"):
    _wadd(_n, 32)                         # out padded to 32 (bank fully written)
for _n in ("f2", "tau2n"):
    _wadd(_n, 16)
for _n in ("zero", "btp", "bt2"):        # full-width strip-0 blocks
    _wadd(_n, 128)
_wadd("bias", 6)                          # 3 fp32 bias cols (bh1,bT1,bf1) bitcast
WCOLS = _CUR[0]


import os
KSTAGE = int(os.environ.get("KSTAGE", "99"))


def build_nc():
    nc = bacc.Bacc("TRN2", target_bir_lowering=False, debug=False,
                   num_devices=NCORES)
    xe_d = nc.dram_tensor("xe", [128, 2 * F], BF16, kind="ExternalInput")
    wt_d = nc.dram_tensor("wt", [128, WCOLS], BF16, kind="ExternalInput")
    out_d = nc.dram_tensor("vout", [128, 5 * F], BF16, kind="ExternalOutput")

    with tile.TileContext(nc) as tc, ExitStack() as ctx:
        sb = ctx.enter_context(tc.tile_pool(name="sb", bufs=1))
        ps = ctx.enter_context(tc.tile_pool(name="ps", bufs=1, space="PSUM"))

        wt = sb.tile([128, WCOLS], BF16, tag="wt", name="wt")
        xe = sb.tile([128, 2 * F], BF16, tag="xe", name="xe")
        junk_src = sb.tile([128, F], BF16, tag="jsrc", name="junk_src")
        a1 = sb.tile([128, F], BF16, tag="a1", name="a1")
        a2c1 = sb.tile([128, F], BF16, tag="a2c1", name="a2c1")
        a2c2 = sb.tile([128, F], BF16, tag="a2c2", name="a2c2")
        th = sb.tile([128, F], BF16, tag="th", name="th")
        a3 = sb.tile([128, F], BF16, tag="a3", name="a3")
        tf = sb.tile([128, F], BF16, tag="tf", name="tf")
        tpt2 = sb.tile([128, 2 * F], BF16, tag="tpt2", name="tpt2")
        sq_th = sb.tile([128, F], BF16, tag="sq_th", name="sq_th")
        sq_tf = sb.tile([128, F], BF16, tag="sq_tf", name="sq_tf")
        sq_tpt2 = sb.tile([128, 2 * F], BF16, tag="sq_tpt2", name="sq_tpt2")
        argh1 = sb.tile([128, F], BF16, tag="argh1", name="argh1")
        argh2 = sb.tile([128, F], BF16, tag="argh2", name="argh2")
        argA12 = sb.tile([128, 2 * F], BF16, tag="argA12", name="argA12")
        m2 = sb.tile([128, F], BF16, tag="m2", name="m2")
        argH = sb.tile([128, F], BF16, tag="argH", name="argH")
        argP = sb.tile([128, F], BF16, tag="argP", name="argP")
        argHP = sb.tile([128, F], BF16, tag="argHP", name="argHP")
        argF = sb.tile([128, F], BF16, tag="argF", name="argF")
        outt = sb.tile([128, 5 * F], BF16, tag="outt", name="outt")

        bias = {n: wt[:, _WCOL["bias"] + 2 * i:_WCOL["bias"] + 2 * i + 2]
                .bitcast(F32) for i, n in enumerate(("bh1", "bT1", "bf1"))}

        # PSUM: 4 rotating 1-bank slots (tag psA) + two 2-bank slots (big)
        junk_ps = ps.tile([128, 2 * F], F32, tag="big1", bufs=1, name="junk_ps")
        pre1 = ps.tile([128, F], F32, tag="psA", bufs=4, name="pre1")
        pre2c1 = ps.tile([128, F], F32, tag="psA", bufs=4, name="pre2c1")
        pre2c2 = ps.tile([128, F], F32, tag="psA", bufs=4, name="pre2c2")
        q31 = ps.tile([128, 2 * F], F32, tag="big2", bufs=1, name="q31")
        hd = ps.tile([128, F], F32, tag="psA", bufs=4, name="hd")
        thp = ps.tile([128, F], F32, tag="psA", bufs=4, name="thp")
        a3p = ps.tile([128, F], F32, tag="psA", bufs=4, name="a3p")
        tfp = ps.tile([128, F], F32, tag="psA", bufs=4, name="tfp")
        tpt2p = ps.tile([128, 2 * F], F32, tag="big1", bufs=1, name="tpt2p")
        bkp = ps.tile([128, F], F32, tag="psA", bufs=4, name="bkp")
        rp = ps.tile([128, F], F32, tag="psA", bufs=4, name="rp")
        q2p = ps.tile([128, F], F32, tag="psA", bufs=4, name="q2p")
        q4p = ps.tile([128, F], F32, tag="psA", bufs=4, name="q4p")
        sp = ps.tile([128, F], F32, tag="psA", bufs=4, name="sp")
        ub = ps.tile([128, 2 * F], F32, tag="big2", bufs=1, name="ub")

        # ---- input DMAs (HWDGE on sync + scalar queues) ----
        nc.sync.dma_start(out=xe, in_=xe_d.ap())
        nc.scalar.dma_start(out=wt, in_=wt_d.ap())


        x17 = lambda i: xe[32 * i:32 * i + 17, 0:F]
        x16 = lambda i: xe[32 * i:32 * i + 16, 0:F]
        e17 = lambda i: xe[32 * i:32 * i + 17, F:2 * F]
        e16 = lambda i: xe[32 * i:32 * i + 16, F:2 * F]

        def wblk(name, i, rows, cols):
            c = _WCOL[name]
            return wt[32 * i:32 * i + rows, c:c + cols]

        def mmv(out_tile, ocols, wname, rhs_fn, rows, rho_in, rho_out, out_p,
                start, stop):
            """One matvec = 4 concurrent sub-array matmuls (one per lane)."""
            for L in range(LANES):
                i = (L + rho_in) % LANES
                j = (L + rho_out) % LANES
                nc.tensor.matmul(
                    out_tile[32 * j:32 * j + out_p, ocols],
                    wblk(wname, i, rows, out_p), rhs_fn(i),
                    start=start, stop=stop, tile_position=(32 * i, 32 * j),
                    skip_group_check=True)

        def act_rhs(t, rho_in, c0=0):
            return lambda i: t[32 * i:32 * i + 32, c0:c0 + F]

        def zmm(out_tile, ocols, wname, start, stop):
            """Full-width strip-0 matmul: clears the bank, writes bias/zero."""
            nc.tensor.matmul(out_tile[0:128, ocols], wblk(wname, 0, 17, 128),
                             x17(0), start=start, stop=stop,
                             tile_position=(0, 0), skip_group_check=True)

        cF = slice(0, F)
        cB = slice(F, 2 * F)

        def fill(bank):
            # full-array junk MM into a bank that is overwritten later;
            # keeps the PE HAM window busy through act/DVE gaps
            nc.tensor.matmul(bank[0:128, 0:F], junk_src[:, 0:128], junk_src,
                             start=True, stop=True, tile_position=(0, 0),
                             skip_group_check=True)

        # A1: pre1(fam0) q1b(2) q3(3) first -- critical chain head
        mmv(pre1, cF, "pre1", x17, 17, 0, 0, 32, True, True)
        mmv(q31, cB, "q1n", e16, 16, 0, 2, 32, True, True)
        mmv(q31, cF, "q3", e16, 16, 0, 3, 32, True, True)
        # zero-clears for multi-writer banks (off critical path)
        zmm(pre2c1, cF, "zero", True, False)
        zmm(pre2c2, cF, "zero", True, False)
        zmm(hd, cF, "zero", True, False)
        # A2: q1a(1) q1a2(2) hd1(0)
        mmv(pre2c1, cF, "q1a", e17, 17, 0, 1, 32, False, False)
        mmv(pre2c2, cF, "q1a", e17, 17, 0, 2, 32, False, False)
        mmv(hd, cF, "hd1", x16, 16, 0, 0, 32, False, False)

        nc.scalar.activation(a1, pre1, AF.Tanh)

        if KSTAGE < 2:
            nc.vector.tensor_copy(outt[:, 0:F], a1)
            nc.sync.dma_start(out=out_d.ap()[:, 0:F], in_=outt[:, 0:F])
            return _finish(nc)
        # B: pre2a -> c1(fam1), c2(fam2)
        mmv(pre2c1, cF, "pre2a", act_rhs(a1, 0), 32, 0, 1, 32, False, True)
        mmv(pre2c2, cF, "pre2a", act_rhs(a1, 0), 32, 0, 2, 32, False, True)

        nc.scalar.activation(a2c1, pre2c1, AF.Tanh)
        nc.scalar.activation(a2c2, pre2c2, AF.Tanh)

        if KSTAGE < 3:
            nc.vector.tensor_copy(outt[:, 0:F], a2c1)
            nc.sync.dma_start(out=out_d.ap()[:, 0:F], in_=outt[:, 0:F])
            return _finish(nc)
        # bias clears for tp/t2 halves
        zmm(tpt2p, cF, "btp", True, False)
        zmm(tpt2p, cB, "bt2", True, False)
        if KSTAGE < 32:
            nc.vector.tensor_copy(outt[:, 0:F], a2c2)
            nc.sync.dma_start(out=out_d.ap()[:, 0:F], in_=outt[:, 0:F])
            raise _Early

        # C1: th(3) a3(0) tf(1) hd2n(2)
        mmv(thp, cF, "th", act_rhs(a2c1, 0), 32, 1, 0, 32, True, True)
        mmv(a3p, cF, "a3", act_rhs(a2c1, 0), 32, 1, 0, 32, True, True)
        mmv(tfp, cF, "tf", act_rhs(a2c2, 0), 32, 2, 3, 32, True, True)
        mmv(hd, cF, "hd2n", act_rhs(a2c2, 0), 32, 2, 0, 32, False, True)

        _b = (lambda n: bias[n]) if KSTAGE >= 34 else (lambda n: 0.0)
        nc.scalar.activation(th, thp, AF.Tanh, bias=_b("bh1"))
        if KSTAGE < 33:
            nc.sync.dma_start(out=out_d.ap()[:, 0:F], in_=th)
            raise _Early
        nc.scalar.activation(a3, a3p, AF.Tanh, bias=_b("bT1"))

        nc.gpsimd.tensor_mul(sq_th, th, th)
        nc.vector.scalar_tensor_tensor(argh1, sq_th, -1.0, hd, ALU.add,
                                       ALU.mult)

        if KSTAGE < 40:
            nc.sync.dma_start(out=out_d.ap()[:, 0:F], in_=argh1)
            return _finish(nc)
        # D: ppsi1(2) ppsi2(3) t2m(1) bk(0)
        mmv(tpt2p, cF, "ppsi1", act_rhs(a3, 0), 32, 0, 3, 32, False, False)
        mmv(tpt2p, cF, "ppsi2", act_rhs(th, 0), 32, 0, 3, 32, False, True)
        mmv(tpt2p, cB, "pre2a", act_rhs(a3, 0), 32, 0, 2, 32, False, True)
        if KSTAGE >= 42:
            mmv(bkp, cF, "bk", act_rhs(argh1, 0), 32, 0, 0, 32, True, True)

        nc.scalar.activation(tpt2[:, F:2 * F], tpt2p[:, F:2 * F], AF.Tanh)
        nc.scalar.activation(tpt2[:, 0:F], tpt2p[:, 0:F], AF.Tanh)
        nc.scalar.activation(tf, tfp, AF.Tanh, bias=_b("bf1"))
        if KSTAGE < 42:
            nc.sync.dma_start(out=out_d.ap()[:, 0:F], in_=tpt2[:, 0:F])
            raise _Early
        nc.gpsimd.tensor_mul(sq_tf, tf, tf)
        nc.vector.tensor_mul(sq_tpt2[:, F:2 * F], tpt2[:, F:2 * F],
                             tpt2[:, F:2 * F])
        if KSTAGE < 43:
            nc.sync.dma_start(out=out_d.ap()[:, 0:F], in_=sq_tpt2[:, 0:F])
            raise _Early
        nc.vector.scalar_tensor_tensor(argA12[:, F:2 * F], sq_tpt2[:, F:2 * F],
                                       -1.0, q31[:, F:2 * F], ALU.add, ALU.mult)
        nc.vector.tensor_mul(m2, tpt2[:, F:2 * F], argA12[:, F:2 * F])
        nc.vector.tensor_mul(sq_tpt2[:, 0:F], tpt2[:, 0:F], tpt2[:, 0:F])
        nc.vector.scalar_tensor_tensor(argA12[:, 0:F], sq_tpt2[:, 0:F],
                                       -1.0, q31[:, 0:F], ALU.add, ALU.mult)
        if KSTAGE < 44:
            nc.sync.dma_start(out=out_d.ap()[:, 0:F], in_=argA12[:, 0:F])
            raise _Early
        nc.vector.scalar_tensor_tensor(argh2, sq_th, -1.0, bkp, ALU.add,
                                       ALU.mult)


        if KSTAGE < 50:
            nc.vector.tensor_copy(outt[:, 0:F], argA12[:, 0:F])
            nc.sync.dma_start(out=out_d.ap()[:, 0:F], in_=outt[:, 0:F])
            return _finish(nc)
        # E1: r(3) q2(1) u1(0)
        mmv(rp, cF, "r", act_rhs(tpt2, 0), 32, 3, 2, 32, True, True)
        mmv(q2p, cF, "tf", act_rhs(argA12, 0, F), 32, 2, 3, 32, True, True)
        mmv(ub, cF, "glin", act_rhs(argh2, 0), 32, 0, 0, 32, True, True)

        nc.vector.scalar_tensor_tensor(outt[:, 3 * F:4 * F], sq_tf, -1.0,
                                       q2p, ALU.add, ALU.mult)
        nc.vector.tensor_mul(argH, m2, rp)

        # C2: s(3)
        mmv(sp, cF, "s", act_rhs(a2c1, 0), 32, 1, 0, 32, True, True)
        nc.scalar.activation(outt[:, 2 * F:3 * F], sp, AF.Tanh)
        nc.sync.dma_start(out=out_d.ap()[:, 2 * F:3 * F],
                          in_=outt[:, 2 * F:3 * F])

        if KSTAGE < 60:
            nc.sync.dma_start(out=out_d.ap()[:, F:2 * F], in_=outt[:, F:2 * F])
            return _finish(nc)
        # E2: q4(3)
        mmv(q4p, cF, "r", act_rhs(argA12, 0), 32, 3, 2, 32, True, True)

        nc.vector.scalar_tensor_tensor(argP, sq_tpt2[:, F:2 * F], -1.0, q4p,
                                       ALU.add, ALU.mult)
        nc.vector.scalar_tensor_tensor(outt[:, 4 * F:5 * F], argH, 2.0, argP,
                                       ALU.mult, ALU.add)
        nc.sync.dma_start(out=out_d.ap()[:, 3 * F:5 * F],
                          in_=outt[:, 3 * F:5 * F])
        nc.vector.tensor_copy(outt[:, 0:F], ub[:, 0:F])
        nc.vector.tensor_mul(outt[:, F:2 * F], outt[:, 2 * F:3 * F], ub[:, 0:F])
        nc.sync.dma_start(out=out_d.ap()[:, 0:2 * F], in_=outt[:, 0:2 * F])


    nc.compile()
    return nc


def _finish(nc):
    return nc


def _host_weights(Wf1, bf1, Wf2, Wh1, bh1, Wh2, WT1, bT1, WT2,
                  Wtau1, btau1, Wtau2, Wpsi1, bpsi1, Wpsi2, WP):
    f = np.float64
    A = lambda a: np.asarray(a, f)
    Wf1, bf1, Wf2 = A(Wf1), A(bf1), A(Wf2)
    Wh1, bh1, Wh2 = A(Wh1), A(bh1), A(Wh2)
    WT1, bT1, WT2 = A(WT1), A(bT1), A(WT2)
    Wtau1, btau1, Wtau2 = A(Wtau1), A(btau1), A(Wtau2)
    Wpsi1, bpsi1, Wpsi2, WP = A(Wpsi1), A(bpsi1), A(Wpsi2), A(WP)
    Wpsi1z, Wpsi1y = Wpsi1[:, :NZ], Wpsi1[:, NZ:]

    def wb(WT_, b=None):                      # lhsT block (+ bias row)
        blk = WT_
        if b is not None:
            blk = np.concatenate([WT_, b.reshape(1, -1)], axis=0)
        return blk

    blocks = {
        "pre1": wb(WT1.T, bT1), "q1a": wb(Wtau1.T, btau1), "q1n": Wtau1.T,
        "q3": Wpsi1z.T, "hd1": Wh1.T, "pre2a": (Wtau1 @ WT2).T,
        "th": (Wh1 @ Wtau2).T, "hd2n": -(Wh1 @ Wtau2).T,
        "a3": (WT1 @ Wtau2).T, "tf": (Wf1 @ Wtau2).T,
        "ppsi1": (Wpsi1z @ WT2).T, "ppsi2": (Wpsi1y @ Wh2).T,
        "bk": Wh2.T @ Wh2 / R, "r": (Wtau1 @ Wpsi2).T,
        "s": np.pad((WP @ Wtau2).T, ((0, 0), (0, 16))),
        "glin": np.pad(Wh1, ((0, 0), (0, 16))),
        "f2": Wf2.T, "tau2n": -Wtau2.T,
    }
    pack = np.zeros((128, WCOLS), np.float32)
    for name, blk in blocks.items():
        c = _WCOL[name]
        rr, cc = blk.shape
        for L in range(LANES):
            pack[32 * L:32 * L + rr, c:c + cc] = blk
    # full-width strip-0 blocks: rows 0-15 zero, row 16 = tiled bias
    for name, bvec in (("zero", np.zeros(128)), ("btp", np.tile(bpsi1, 4)),
                       ("bt2", np.tile(btau1, 4))):
        pack[16, _WCOL[name]:_WCOL[name] + 128] = bvec
    packb = pack.astype(np_bf16)
    for i, bvec in enumerate((bh1, bT1, bf1)):
        col = np.tile(np.asarray(bvec, np.float32), LANES).reshape(128, 1)
        packb[:, _WCOL["bias"] + 2 * i:_WCOL["bias"] + 2 * i + 2] = \
            col.view(np_bf16)
    return packb


_CACHE = {}
_HOST = {}


def _get_nc():
    if "nc" not in _CACHE:
        _CACHE["nc"] = build_nc()
    return _CACHE["nc"]


def _in_maps(x_batch, e_batch, wts):
    wpack = _host_weights(**wts)
    _HOST["Wf2"] = np.asarray(wts["Wf2"], np.float64)
    _HOST["Wtau2"] = np.asarray(wts["Wtau2"], np.float64)

    def strips(a, rows):
        # (2048, rows) -> [128, 512] with lane L at partitions 32L..32L+rows,
        # row 16 = ones, rest zero
        out = np.zeros((128, F), np.float32)
        a = np.asarray(a, np.float32).reshape(LANES, F, rows)
        for L in range(LANES):
            out[32 * L:32 * L + rows] = a[L].T
            out[32 * L + 16] = 1.0
        return out

    in_maps = []
    for c in range(NCORES):
        cs = slice(c * PER_CORE, (c + 1) * PER_CORE)
        xeb = np.concatenate([strips(x_batch[cs], N), strips(e_batch[cs], NZ)],
                             axis=1).astype(np_bf16)
        in_maps.append({"xe": np.ascontiguousarray(xeb), "wt": wpack})
    return in_maps


def _reduce(results):
    Wf2, Wtau2 = _HOST["Wf2"], _HOST["Wtau2"]
    total = np.float64(0.0)
    for r in results:
        o = np.asarray(r["vout"], np.float64)      # (128, 2560)
        o = o.reshape(LANES, 32, 5 * F)
        u = o[:, 0:16, 0:F]
        m = o[:, 0:16, F:2 * F]
        s = o[:, 0:16, 2 * F:3 * F]
        aF = o[:, :, 3 * F:4 * F]                  # (L, 32, F)
        aHP = o[:, :, 4 * F:5 * F]
        v = (u + s * m.sum(axis=1, keepdims=True)
             + np.einsum('nh,lhc->lnc', Wf2, aF)
             - np.einsum('nh,lhc->lnc', Wtau2, aHP))
        total += np.sqrt((v * v).sum(axis=1)).sum()
    return np.asarray(total / B, dtype=np.float32)


def kernel(x_batch, e_batch, **wts):
    from concourse.bass_utils import run_bass_kernel_spmd
    nc = _get_nc()
    in_maps = _in_maps(np.asarray(x_batch, np.float32),
                       np.asarray(e_batch, np.float32), wts)
    res = run_bass_kernel_spmd(nc, in_maps, core_ids=list(range(NCORES)))
    return _reduce(res.results)


if __name__ == "__main__":
    rng = np.random.default_rng(0)
    wts = {
        "Wf1": rng.normal(size=(H, N)) * .3, "bf1": rng.normal(size=(H,)) * .3,
        "Wf2": rng.normal(size=(N, H)) * .3,
        "Wh1": rng.normal(size=(H, N)) * .3, "bh1": rng.normal(size=(H,)) * .3,
        "Wh2": rng.normal(size=(OUT, H)) * .3,
        "WT1": rng.normal(size=(H, N)) * .3, "bT1": rng.normal(size=(H,)) * .3,
        "WT2": rng.normal(size=(NZ, H)) * .3,
        "Wtau1": rng.normal(size=(H, NZ)) * .3,
        "btau1": rng.normal(size=(H,)) * .3,
        "Wtau2": rng.normal(size=(N, H)) * .3,
        "Wpsi1": rng.normal(size=(H, NZ + OUT)) * .3,
        "bpsi1": rng.normal(size=(H,)) * .3,
        "Wpsi2": rng.normal(size=(NZ, H)) * .3,
        "WP": rng.normal(size=(N, N)) * .3,
    }
    x = rng.normal(size=(B, N)).astype(np.float32)
    e = (rng.normal(size=(B, NZ)) * 0.1).astype(np.float32)
    print(kernel(x, e, **{k: np.asarray(v, np.float32)
                          for k, v in wts.items()}))
